# revision 10
# baseline (speedup 1.0000x reference)
"""Trainium2 Bass kernel for nn_DQSN (spiking DQN head).

Math: I1 = x @ W1.T + b1 is constant across the T=16 IF-neuron steps and the
IF neuron hard-resets to exactly 0, so each element's spike train is periodic
with period k = ceil(1/c) (c = I1 element); the LIF readout collapses to
    v2 = Z @ W2.T + b2*(1 - 2^-16),   Z(c) = zval[k(c)]
a piecewise-constant staircase in c with 16 breakpoints at 1/k. Verified
bit-compatible with the iterative float32 reference (0/67M spike flips).

Device mapping (8 cores, batch-parallel, 2048 rows each, transposed layout
[h partitions x b free]):
  - PE: I1 via fp32 matmul (K=5, bias folded in via ones-row);
        staircase sum via 16 per-term fp32r matmuls with d_k-scaled W2
        accumulated in PSUM, 4-way col-group packed (M=2 each).
  - DVE/ACT/GPSIMD: 16 threshold compares (is_ge / Sign) split across
        engines.
"""

import numpy as np
from contextlib import ExitStack

T = 16
B_FULL, XD, H = 16384, 4, 4096
NCORES = 8
BC = B_FULL // NCORES          # 2048 batch rows per core
PCH = 128                      # partition chunk of H
WSUM = float(sum(2.0 ** (t - T) for t in range(T)))  # 1 - 2^-16

# term i (0..15) compares c >= 1/(i+1)
ACT_IDX = (2, 6, 10, 14)       # computed on ScalarE via Sign
GP_IDX = (1, 5, 9, 13)         # computed on GPSIMD
# remaining 8 on VectorE


def _tables():
    zval = np.zeros(18, np.float64)
    for k in range(1, 17):
        m = 16 // k
        zval[k] = sum(2.0 ** (j * k - 17) for j in range(1, m + 1))
    bks = (np.float32(1.0) / np.arange(1, 17, dtype=np.float32)).astype(np.float32)
    dks = np.array([zval[k] - zval[k + 1] for k in range(1, 17)], np.float32)
    return bks, dks


def _split_drain_waits(nc, caps=None):
    """walrus codegen accepts only a small number of sem waits per
    instruction (varies by opcode); move excess waits onto preceding
    same-engine NOPs."""
    import copy
    import bass_rust

    caps = caps or {}
    default_cap = 1

    # template nop (created on DVE, engine overwritten per use)
    tmpl = nc.vector.nop().ins
    bb_cur = None
    for bb in nc.m.functions[0].blocks:
        if tmpl in bb.instructions:
            bb.instructions.remove(tmpl)
    nidx = 0
    for bb in nc.m.functions[0].blocks:
        il = bb.instructions
        newlist = []
        for ins in il:
            si = ins.sync_info
            w = list(si.on_wait or []) if si is not None else []
            cap = caps.get(type(ins).__name__, default_cap)
            if len(w) > cap:
                chunks = [w[j : j + cap] for j in range(0, len(w), cap)]
                for ch in chunks[:-1]:
                    d = copy.deepcopy(tmpl)
                    d.name = f"I-waitnop-{nidx}"
                    nidx += 1
                    d.engine = ins.engine
                    d.sync_info = bass_rust.SyncInfo(on_wait=ch, on_update=[])
                    nc.register_instruction(d)
                    newlist.append(d)
                si.on_wait = chunks[-1]
                ins.sync_info = si
            newlist.append(ins)
        il[:] = newlist


def build_kernel(bc=BC, h=H, split_drains=True):
    """Build the per-core Bass program (identical on all cores)."""
    from concourse import bass, tile, mybir

    bks, dks = _tables()
    nch = h // PCH
    nsub = bc // 512
    f32 = mybir.dt.float32
    f32r = mybir.dt.float32r
    ge = mybir.AluOpType.is_ge

    nc = bass.Bass("TRN2", target_bir_lowering=False, debug=False,
                   num_devices=NCORES)
    xa_d = nc.declare_dram_parameter("xa", [XD + 1, bc], f32, isOutput=False)
    w1b_d = nc.declare_dram_parameter("w1b", [XD + 1, h], f32, isOutput=False)
    wkt_d = nc.declare_dram_parameter("wkt", [PCH, nch * 32], f32r, isOutput=False)
    bias_d = nc.declare_dram_parameter("bias2", [2, 1], f32, isOutput=False)
    out_d = nc.declare_dram_parameter("out", [2, bc], f32, isOutput=True)

    with tile.TileContext(nc) as tc:
        with ExitStack() as ctx:
            cpool = ctx.enter_context(tc.tile_pool(name="const", bufs=1))
            csb = ctx.enter_context(tc.tile_pool(name="csb", bufs=3))
            spool = ctx.enter_context(tc.tile_pool(name="s", bufs=8))
            fpool = ctx.enter_context(tc.tile_pool(name="fin", bufs=8))
            pc = ctx.enter_context(
                tc.tile_pool(name="pc", bufs=1, space="PSUM"))
            pv = ctx.enter_context(
                tc.tile_pool(name="pv", bufs=1, space="PSUM"))

            xa = cpool.tile([XD + 1, bc], f32, tag="xa")
            w1b = cpool.tile([XD + 1, h], f32, tag="w1b")
            wkt = cpool.tile([PCH, nch * 32], f32r, tag="wkt")
            bias2 = cpool.tile([2, 1], f32, tag="bias2")
            bias_act = cpool.tile([PCH, len(ACT_IDX)], f32, tag="bias_act")
            for pos, i in enumerate(ACT_IDX):
                nc.gpsimd.memset(bias_act[:, pos : pos + 1], -float(bks[i]))
            nc.sync.dma_start(xa[:], xa_d[:])
            nc.sync.dma_start(w1b[:], w1b_d[:])
            nc.sync.dma_start(wkt[:], wkt_d[:])
            nc.sync.dma_start(bias2[:], bias_d[:])

            # persistent PSUM accumulators: one bank per 512-col subtile;
            # col-group g owns partitions [32g, 32g+2)
            v2ps = [pv.tile([PCH, 512], f32, tag=f"v2ps{n}", name=f"v2ps{n}")
                    for n in range(nsub)]

            for j in range(nch):
                c_ps = pc.tile([PCH, bc], f32, tag="c_ps")
                for n in range(nsub):
                    nc.tensor.matmul(
                        c_ps[:, 512 * n : 512 * (n + 1)],
                        w1b[:, PCH * j : PCH * (j + 1)],
                        xa[:, 512 * n : 512 * (n + 1)],
                        start=True, stop=True)
                c_sb = csb.tile([PCH, bc], f32, tag="c_sb")
                nc.scalar.activation(c_sb[:], c_ps[:],
                                     mybir.ActivationFunctionType.Copy)

                for i in range(16):
                    s_t = spool.tile([PCH, bc], f32r, tag="s")
                    if i in ACT_IDX:
                        nc.scalar.activation(
                            s_t[:], c_sb[:],
                            mybir.ActivationFunctionType.Sign,
                            bias=bias_act[:, ACT_IDX.index(i) : ACT_IDX.index(i) + 1])
                    elif i in GP_IDX:
                        nc.gpsimd.tensor_scalar(
                            s_t[:], c_sb[:], float(bks[i]), None, ge)
                    else:
                        nc.vector.tensor_scalar(
                            s_t[:], c_sb[:], float(bks[i]), None, ge)
                    lhs = wkt[:, 32 * j + 2 * i : 32 * j + 2 * i + 2]
                    for n in range(nsub):
                        nc.tensor.matmul(
                            v2ps[n][0:2, :],
                            lhs,
                            s_t[:, 512 * n : 512 * (n + 1)],
                            start=(j == 0 and i == 0),
                            stop=(j == nch - 1 and i == 15),
                            skip_group_check=True)

            # tail: fold 4 col-groups + bias, emit [2, bc]
            out_sb = fpool.tile([2, bc], f32, tag="out_sb")
            for n in range(nsub):
                nc.scalar.activation(
                    out_sb[:, 512 * n : 512 * (n + 1)], v2ps[n][0:2, :],
                    mybir.ActivationFunctionType.Identity,
                    bias=bias2[:])
            nc.sync.dma_start(out_d[:], out_sb[:])

    if split_drains:
        _split_drain_waits(nc)
    return nc


def host_inputs(x, W1, b1, W2, b2, bc=BC, h=H):
    """Per-core input maps (numpy prep of small tensors only)."""
    bks, dks = _tables()
    nch = h // PCH
    x = np.ascontiguousarray(x, np.float32)
    W1 = np.asarray(W1, np.float32)
    b1 = np.asarray(b1, np.float32)
    W2 = np.asarray(W2, np.float32)
    b2 = np.asarray(b2, np.float32)

    w1b = np.concatenate([W1.T, b1[None, :]], axis=0)          # [5, H]
    scales = dks.copy()
    scales[list(ACT_IDX)] *= 0.5
    # wkt[p, j*32 + 2i + jj] = scales[i] * W2[jj, j*128+p]
    wkt = np.empty((PCH, nch * 32), np.float32)
    for j in range(nch):
        blk = W2[:, j * PCH : (j + 1) * PCH]                   # [2, 128]
        w = blk.T[:, None, :] * scales[:, None]                # [128, 16, 2]
        wkt[:, 32 * j : 32 * (j + 1)] = w.reshape(PCH, 32)
    bias_corr = sum(0.5 * dks[i] for i in ACT_IDX)
    bias2 = (b2 * WSUM + bias_corr * W2.sum(axis=1)).astype(np.float32)

    ncores = x.shape[0] // bc
    ones = np.ones((1, bc), np.float32)
    maps = []
    for c in range(ncores):
        xs = x[c * bc : (c + 1) * bc]
        xa = np.concatenate([np.ascontiguousarray(xs.T), ones], axis=0)
        maps.append({
            "xa": np.ascontiguousarray(xa, np.float32),
            "w1b": np.ascontiguousarray(w1b, np.float32),
            "wkt": np.ascontiguousarray(wkt, np.float32),
            "bias2": np.ascontiguousarray(bias2.reshape(2, 1)),
        })
    return maps


_cached = {}


def kernel(x, W1, b1, W2, b2):
    from concourse.bass_utils import run_bass_kernel_spmd

    if "nc" not in _cached:
        _cached["nc"] = build_kernel()
    nc = _cached["nc"]
    in_maps = host_inputs(x, W1, b1, W2, b2)
    core_ids = list(range(NCORES))
    res = run_bass_kernel_spmd(nc, in_maps, core_ids)
    outs = [np.asarray(res.results[i]["out"]).T for i in range(NCORES)]
    return np.ascontiguousarray(np.concatenate(outs, axis=0), np.float32)


# revision 11
# speedup vs baseline: 8.0773x; 8.0773x over previous
"""Trainium2 Bass kernel for nn_DQSN (spiking DQN head).

Math: I1 = x @ W1.T + b1 is constant across the T=16 IF-neuron steps and the
IF neuron hard-resets to exactly 0, so each element's spike train is periodic
with period k = ceil(1/c) (c = I1 element); the LIF readout collapses to
    v2 = Z @ W2.T + b2*(1 - 2^-16),   Z(c) = zval[k(c)]
a piecewise-constant staircase in c with 16 breakpoints at 1/k. Verified
bit-compatible with the iterative float32 reference (0/67M spike flips).

Device mapping (8 cores, batch-parallel, 2048 rows each, transposed layout
[h partitions x b free]):
  - PE: I1 via fp16 hi/lo-split matmul (K=14, error ~2^-22, bias folded);
        staircase sum via 16 per-term fp16 matmuls (S in {0,1} exact in
        fp16; d_k-scaled W2 at fp16 precision) accumulated in PSUM.
  - DVE (is_ge) + ACT (Sign) produce the 16 threshold masks.
"""

import numpy as np
from contextlib import ExitStack

T = 16
B_FULL, XD, H = 16384, 4, 4096
NCORES = 8
BC = B_FULL // NCORES          # 2048 batch rows per core
PCH = 128                      # partition chunk of H
KI1 = 3 * XD + 2               # whi*xhi + whi*xlo + wlo*xhi + b1hi + b1lo
WSUM = float(sum(2.0 ** (t - T) for t in range(T)))  # 1 - 2^-16

# term i (0..15) compares c >= 1/(i+1)
ACT_IDX = (2, 5, 8, 11, 14)    # computed on ScalarE via Sign
GP_IDX = ()                    # gpsimd tensor_scalar measured ~31.8us -> unused
# remaining 11 on VectorE


def _tables():
    zval = np.zeros(18, np.float64)
    for k in range(1, 17):
        m = 16 // k
        zval[k] = sum(2.0 ** (j * k - 17) for j in range(1, m + 1))
    bks = (np.float32(1.0) / np.arange(1, 17, dtype=np.float32)).astype(np.float32)
    dks = np.array([zval[k] - zval[k + 1] for k in range(1, 17)], np.float32)
    return bks, dks


def _split_drain_waits(nc, caps=None):
    """walrus codegen accepts only one sem wait per instruction here; move
    excess waits onto preceding same-engine NOPs."""
    import copy
    import bass_rust

    caps = caps or {}
    default_cap = 1

    tmpl = nc.vector.nop().ins
    for bb in nc.m.functions[0].blocks:
        if tmpl in bb.instructions:
            bb.instructions.remove(tmpl)
    nidx = 0
    for bb in nc.m.functions[0].blocks:
        il = bb.instructions
        newlist = []
        for ins in il:
            si = ins.sync_info
            w = list(si.on_wait or []) if si is not None else []
            cap = caps.get(type(ins).__name__, default_cap)
            if len(w) > cap:
                chunks = [w[j : j + cap] for j in range(0, len(w), cap)]
                for ch in chunks[:-1]:
                    d = copy.deepcopy(tmpl)
                    d.name = f"I-waitnop-{nidx}"
                    nidx += 1
                    d.engine = ins.engine
                    d.sync_info = bass_rust.SyncInfo(on_wait=ch, on_update=[])
                    nc.register_instruction(d)
                    newlist.append(d)
                si.on_wait = chunks[-1]
                ins.sync_info = si
            newlist.append(ins)
        il[:] = newlist


def _dedup_ldweights(nc):
    """Consecutive PE matmuls with the same weights AP reload the PE array
    each time (~120ns); mark repeats as non-self-loading."""
    for bb in nc.m.functions[0].blocks:
        last_w = None
        for ins in bb.instructions:
            if type(ins).__name__ != "InstMatmult":
                continue
            try:
                w = ins.ins[1]
                sig = str(w)
            except Exception:
                last_w = None
                continue
            if last_w is not None and sig == last_w and ins.ldweights:
                ins.ldweights = False
            else:
                last_w = sig


def build_kernel(bc=BC, h=H, split_drains=True, dedup_ldw=True):
    """Build the per-core Bass program (identical on all cores)."""
    from concourse import bass, tile, mybir

    bks, dks = _tables()
    nch = h // PCH
    nsub = bc // 512
    f32 = mybir.dt.float32
    f16 = mybir.dt.float16
    ge = mybir.AluOpType.is_ge

    nc = bass.Bass("TRN2", target_bir_lowering=False, debug=False,
                   num_devices=NCORES)
    xa_d = nc.declare_dram_parameter("xa", [KI1, bc], f16, isOutput=False)
    w1b_d = nc.declare_dram_parameter("w1b", [KI1, h], f16, isOutput=False)
    wkt_d = nc.declare_dram_parameter("wkt", [PCH, nch * 32], f16, isOutput=False)
    bias_d = nc.declare_dram_parameter("bias2", [2, 1], f32, isOutput=False)
    out_d = nc.declare_dram_parameter("out", [2, bc], f32, isOutput=True)

    with tile.TileContext(nc) as tc:
        with ExitStack() as ctx:
            cpool = ctx.enter_context(tc.tile_pool(name="const", bufs=1))
            csb = ctx.enter_context(tc.tile_pool(name="csb", bufs=3))
            spool = ctx.enter_context(tc.tile_pool(name="s", bufs=10))
            fpool = ctx.enter_context(tc.tile_pool(name="fin", bufs=2))
            pc = ctx.enter_context(
                tc.tile_pool(name="pc", bufs=1, space="PSUM"))
            pv = ctx.enter_context(
                tc.tile_pool(name="pv", bufs=1, space="PSUM"))

            xa = cpool.tile([KI1, bc], f16, tag="xa")
            w1b = cpool.tile([KI1, h], f16, tag="w1b")
            wkt = cpool.tile([PCH, nch * 32], f16, tag="wkt")
            bias2 = cpool.tile([2, 1], f32, tag="bias2")
            bias_act = cpool.tile([PCH, len(ACT_IDX)], f32, tag="bias_act")
            for pos, i in enumerate(ACT_IDX):
                nc.gpsimd.memset(bias_act[:, pos : pos + 1], -float(bks[i]))
            nc.sync.dma_start(xa[:], xa_d[:])
            nc.sync.dma_start(w1b[:], w1b_d[:])
            nc.sync.dma_start(wkt[:], wkt_d[:])
            nc.sync.dma_start(bias2[:], bias_d[:])

            v2ps = [pv.tile([PCH, 512], f32, tag=f"v2ps{n}", name=f"v2ps{n}")
                    for n in range(nsub)]

            for j in range(nch):
                c_ps = pc.tile([PCH, bc], f32, tag="c_ps")
                for n in range(nsub):
                    nc.tensor.matmul(
                        c_ps[:, 512 * n : 512 * (n + 1)],
                        w1b[:, PCH * j : PCH * (j + 1)],
                        xa[:, 512 * n : 512 * (n + 1)],
                        start=True, stop=True)
                c_sb = csb.tile([PCH, bc], f32, tag="c_sb")
                nc.scalar.activation(c_sb[:], c_ps[:],
                                     mybir.ActivationFunctionType.Copy)

                for i in range(16):
                    s_t = spool.tile([PCH, bc], f16, tag="s")
                    if i in ACT_IDX:
                        p = ACT_IDX.index(i)
                        nc.scalar.activation(
                            s_t[:], c_sb[:],
                            mybir.ActivationFunctionType.Sign,
                            bias=bias_act[:, p : p + 1])
                    else:
                        nc.vector.tensor_scalar(
                            s_t[:], c_sb[:], float(bks[i]), None, ge)
                    lhs = wkt[:, 32 * j + 2 * i : 32 * j + 2 * i + 2]
                    for n in range(nsub):
                        nc.tensor.matmul(
                            v2ps[n][0:2, :],
                            lhs,
                            s_t[:, 512 * n : 512 * (n + 1)],
                            start=(j == 0 and i == 0),
                            stop=(j == nch - 1 and i == 15),
                            skip_group_check=True)

            # tail: add bias, emit [2, bc]
            out_sb = fpool.tile([2, bc], f32, tag="out_sb")
            for n in range(nsub):
                nc.scalar.activation(
                    out_sb[:, 512 * n : 512 * (n + 1)], v2ps[n][0:2, :],
                    mybir.ActivationFunctionType.Identity,
                    bias=bias2[:])
            nc.sync.dma_start(out_d[:], out_sb[:])

    if dedup_ldw:
        _dedup_ldweights(nc)
    if split_drains:
        _split_drain_waits(nc)
    return nc


def _f16_split(a):
    hi = a.astype(np.float16)
    lo = (a.astype(np.float32) - hi.astype(np.float32)).astype(np.float16)
    return hi, lo


def host_inputs(x, W1, b1, W2, b2, bc=BC, h=H):
    """Per-core input maps (numpy prep of small tensors only)."""
    bks, dks = _tables()
    nch = h // PCH
    x = np.asarray(x, np.float32)
    W1 = np.asarray(W1, np.float32)
    b1 = np.asarray(b1, np.float32)
    W2 = np.asarray(W2, np.float32)
    b2 = np.asarray(b2, np.float32)

    whi, wlo = _f16_split(W1.T)            # [4, H]
    bhi, blo = _f16_split(b1[None, :])     # [1, H]
    # K rows: whi (pairs xhi), whi (pairs xlo), wlo (pairs xhi), bhi, blo
    w1b = np.concatenate([whi, whi, wlo, bhi, blo], axis=0)  # [14, H] fp16

    scales = dks.copy()
    scales[list(ACT_IDX)] *= 0.5
    wkt = np.empty((PCH, nch * 32), np.float16)
    for j in range(nch):
        blk = W2[:, j * PCH : (j + 1) * PCH]                 # [2, 128]
        w = (blk.T[:, None, :] * scales[:, None]).astype(np.float16)
        wkt[:, 32 * j : 32 * (j + 1)] = w.reshape(PCH, 32)
    bias_corr = sum(0.5 * dks[i] for i in ACT_IDX)
    bias2 = (b2 * WSUM + bias_corr * W2.sum(axis=1)).astype(np.float32)

    ncores = x.shape[0] // bc
    ones = np.ones((1, bc), np.float16)
    maps = []
    for c in range(ncores):
        xs = x[c * bc : (c + 1) * bc]                        # [bc, 4]
        xhi, xlo = _f16_split(np.ascontiguousarray(xs.T))    # [4, bc]
        xa = np.concatenate([xhi, xlo, xhi, ones, ones], axis=0)  # [14, bc]
        maps.append({
            "xa": np.ascontiguousarray(xa, np.float16),
            "w1b": np.ascontiguousarray(w1b, np.float16),
            "wkt": np.ascontiguousarray(wkt, np.float16),
            "bias2": np.ascontiguousarray(bias2.reshape(2, 1), np.float32),
        })
    return maps


_cached = {}


def kernel(x, W1, b1, W2, b2):
    from concourse.bass_utils import run_bass_kernel_spmd

    if "nc" not in _cached:
        _cached["nc"] = build_kernel()
    nc = _cached["nc"]
    in_maps = host_inputs(x, W1, b1, W2, b2)
    core_ids = list(range(NCORES))
    res = run_bass_kernel_spmd(nc, in_maps, core_ids)
    outs = [np.asarray(res.results[i]["out"]).T for i in range(NCORES)]
    return np.ascontiguousarray(np.concatenate(outs, axis=0), np.float32)
